# revision 14
# baseline (speedup 1.0000x reference)
"""Trainium2 Bass kernel for nn_DebiasIntraDist (segment_reduce).

Full-input contract: kernel(**inputs) takes the complete (unsharded) inputs
and returns the full scalar loss.

Design (v3):
  * The 2048 (demog, label) groups are host-partitioned into 16 bins of
    exactly 128 groups each, balanced by row count (LPT), and each core
    processes two bins sequentially (2 "passes"). Each pass owns 128 groups,
    so the per-tile one-hot is a single [128, 128] stationary operand and
    every feature tile streams through the PE exactly once.
  * feats are quantized to fp16 on the host - rel error of the final loss
    vs the fp32 reference is ~1.2e-3 (measured on the fixed key-0 data),
    well inside the 2e-2 gate - which halves both DMA bytes and PE cost
    vs fp32, and (unlike bf16) makes a single fp16 column of the row
    sum-of-squares accurate enough, so no error-compensated hi/lo packs
    are needed anywhere.
  * Per-group count / presence / weights depend only on the labels, so they
    are computed on the host and folded into a per-slot weight matrix W:
        intra_d = sum_g W[g, d] * (sumsq_g - |sums_g|^2 / cnt_g)
    The device reduces its 256 groups to a [1, 4] per-demog partial with one
    tiny fp32 matmul per pass; the host sums the 8 cores' partials (32
    floats) and finishes the scalar loss. No collectives at all.
  * Row sum-of-squares work alternates between the Scalar engine
    (activation Square with accumulate) and the Vector+PE pair (x*x
    multiply, one-hot matmul into a second PSUM bank) with a compile-time
    pattern, so no single engine bottlenecks. GpSimd is kept out of the
    per-tile loop entirely (its tensor ops are ~1.7us each - software).
"""

import numpy as np

try:
    import concourse.bacc as bacc
except ImportError:  # fresh environment without PYTHONPATH set up
    import sys
    for p in ("/root/.axon_site/_ro/trn_rl_repo", "/opt/trn_rl_repo",
              "/root/.axon_site/_ro/pypackages"):
        if p not in sys.path:
            sys.path.append(p)
    import concourse.bacc as bacc
import concourse.mybir as mybir
import concourse.tile as tile
import concourse.bass_utils as bass_utils

N_CORES = 8
P = 128
D = 512          # feature dim
G = 2048         # total groups
NB = 16          # group bins (2 per core)
ND = 4           # demog values
CH = 8           # sample-tiles per steady-state feats DMA chunk

# per-(within-pass-tile) row-sumsq path pattern:
#   S = scalar activation square+accum -> fp16 sq column -> tiny PE matmul
#   P = vector x*x multiply -> PE one-hot matmul into a PSUM bank
PAT = "PSPSPSPSPSPSPSP"

_cache: dict[tuple, object] = {}


def _build(T_h: int, pat: str = PAT):
    """Compile the SPMD kernel: 2 passes of T_h tiles, 128 groups each."""
    T2 = 2 * T_h
    fp32 = mybir.dt.float32
    fp16 = mybir.dt.float16
    Alu = mybir.AluOpType
    Act = mybir.ActivationFunctionType

    paths = [pat[j % len(pat)] for j in range(T_h)]
    sq_tiles = [j for j in range(T_h) if paths[j] == "S"]   # feed ps_sm
    x2_tiles = [j for j in range(T_h) if paths[j] == "P"]   # feed ps_x2

    nc = bacc.Bacc("TRN2", target_bir_lowering=False, debug=False,
                   enable_asserts=True, num_devices=N_CORES)

    feats_d = nc.dram_tensor("feats", [P, T2 * D], fp16,
                             kind="ExternalInput").ap()
    labs_d = nc.dram_tensor("labs", [P, T2], fp32, kind="ExternalInput").ap()
    aux_d = nc.dram_tensor("aux", [P, 10], fp32, kind="ExternalInput").ap()
    parts_d = nc.dram_tensor("partials", [1, ND], fp32,
                             kind="ExternalOutput").ap()

    with tile.TileContext(nc) as tc:
        with (
            tc.tile_pool(name="const", bufs=1) as constp,
            tc.tile_pool(name="fx", bufs=4) as fxp,
            tc.tile_pool(name="oh", bufs=4) as ohp,
            tc.tile_pool(name="x2", bufs=4) as x2p,
            tc.tile_pool(name="scr", bufs=3) as scrp,
            tc.tile_pool(name="sq", bufs=4) as sqp,
            tc.tile_pool(name="post", bufs=1) as postp,
            tc.tile_pool(name="ps", bufs=1, space="PSUM") as psp,
        ):
            # constants / inputs (early, on queues that boot fastest)
            iota = constp.tile([P, P], fp16, tag="iota")
            nc.gpsimd.iota(iota[:], [[1, P]], channel_multiplier=0,
                           allow_small_or_imprecise_dtypes=True)
            labs = constp.tile([P, T2], fp32, tag="labs")
            auxs = constp.tile([P, 10], fp32, tag="auxs")

            # per-pass group accumulators (each owns a PSUM bank)
            ps_sums = [psp.tile([P, D], fp32, tag=f"sums{p}", name=f"sums{p}")
                       for p in range(2)]
            ps_x2 = [psp.tile([P, D], fp32, tag=f"x2{p}", name=f"x2{p}")
                     for p in range(2)] if x2_tiles else None
            ps_sm = [psp.tile([P, 1], fp32, tag=f"sm{p}", name=f"sm{p}")
                     for p in range(2)] if sq_tiles else None
            ps4 = psp.tile([1, ND], fp32, tag="p4", name="p4")

            def post_pass(p):
                """Reduce pass p's accumulators to a [1,4] demog partial."""
                ssq = postp.tile([P, 1], fp32, tag=f"ssq{p}")
                if sq_tiles:
                    nc.vector.tensor_copy(out=ssq[:], in_=ps_sm[p][:])
                    if x2_tiles:
                        ssq2 = postp.tile([P, 1], fp32, tag=f"ssq2{p}")
                        nc.vector.tensor_reduce(out=ssq2[:], in_=ps_x2[p][:],
                                                axis=mybir.AxisListType.X,
                                                op=Alu.add)
                        nc.vector.tensor_tensor(out=ssq[:], in0=ssq[:],
                                                in1=ssq2[:], op=Alu.add)
                else:
                    nc.vector.tensor_reduce(out=ssq[:], in_=ps_x2[p][:],
                                            axis=mybir.AxisListType.X,
                                            op=Alu.add)
                scr2 = scrp.tile([P, D], fp16, tag="scr")
                norm2 = postp.tile([P, 1], fp32, tag=f"n2{p}")
                nc.scalar.activation(scr2[:], ps_sums[p][:], Act.Square,
                                     accum_out=norm2[:])
                t1 = postp.tile([P, 1], fp32, tag=f"t1{p}")
                nc.vector.tensor_tensor(out=t1[:], in0=norm2[:],
                                        in1=auxs[:, 8 + p:9 + p], op=Alu.mult)
                val = postp.tile([P, 1], fp32, tag=f"val{p}")
                nc.vector.tensor_tensor(out=val[:], in0=ssq[:], in1=t1[:],
                                        op=Alu.subtract)
                nc.tensor.matmul(out=ps4[:], lhsT=val[:],
                                 rhs=auxs[:, 4 * p:4 * p + 4],
                                 start=(p == 0), stop=(p == 1))

            # chunked feats DMA schedule: ramped small first chunks so tile 0
            # is never queued behind megabytes of steady-state traffic; later
            # chunks are throttled by fx-pool buffer rotation (bufs=4)
            sched = []
            t = 0
            for want in (1, 1, 2, 3):
                if t < T2:
                    L = min(want, T2 - t)
                    sched.append((t, L))
                    t += L
            while t < T2:
                L = min(CH, T2 - t)
                sched.append((t, L))
                t += L

            ci = 0
            n_ramp = sum(1 for _, L in sched if L < CH)
            for t0, L in sched:
                fx = fxp.tile([P, CH * D], fp16, tag="fx")
                # chunk 0 alone on the scalar queue; the other ramp chunks
                # serialize their descriptor generation on the (otherwise
                # idle) gpsimd queue so tile 0's data never contends in the
                # DMA rings; steady chunks go to sync, which is naturally
                # throttled by fx-pool buffer rotation
                if ci == 0:
                    q = nc.scalar
                elif ci < n_ramp:
                    q = nc.gpsimd
                else:
                    q = nc.sync
                q.dma_start(out=fx[:, :L * D],
                            in_=feats_d[:, t0 * D:(t0 + L) * D])
                if ci == 0:
                    # labs right after tile 0 on the scalar queue (needed by
                    # the first is_equal)
                    nc.scalar.dma_start(out=labs[:], in_=labs_d[:])
                if ci == n_ramp - 1:
                    nc.gpsimd.dma_start(out=auxs[:], in_=aux_d[:])
                ci += 1
                for k in range(L):
                    ti = t0 + k
                    p, j = divmod(ti, T_h)
                    path = paths[j]
                    X = fx[:, k * D:(k + 1) * D]

                    if path == "P":
                        x2 = x2p.tile([P, D], fp16, tag="x2")
                        nc.vector.tensor_tensor(out=x2[:], in0=X, in1=X,
                                                op=Alu.mult)
                    oh = ohp.tile([P, P], fp16, tag="oh")
                    nc.vector.tensor_scalar(
                        out=oh[:], in0=iota[:], scalar1=labs[:, ti:ti + 1],
                        scalar2=None, op0=Alu.is_equal,
                    )
                    nc.tensor.matmul(out=ps_sums[p][:], lhsT=oh[:], rhs=X,
                                     start=(j == 0), stop=(j == T_h - 1))

                    if path == "P":
                        nc.tensor.matmul(out=ps_x2[p][:], lhsT=oh[:],
                                         rhs=x2[:],
                                         start=(j == x2_tiles[0]),
                                         stop=(j == x2_tiles[-1]))
                    else:
                        scr = scrp.tile([P, D], fp16, tag="scr")
                        sq = sqp.tile([P, 1], fp32, tag="sq")
                        nc.scalar.activation(scr[:], X, Act.Square,
                                             accum_out=sq[:])
                        sqh = sqp.tile([P, 1], fp16, tag="sqh")
                        nc.vector.tensor_copy(out=sqh[:], in_=sq[:])
                        nc.tensor.matmul(out=ps_sm[p][:], lhsT=oh[:],
                                         rhs=sqh[:],
                                         start=(j == sq_tiles[0]),
                                         stop=(j == sq_tiles[-1]))

                    if j == T_h - 1:
                        post_pass(p)

            part = postp.tile([1, ND], fp32, tag="part")
            nc.vector.tensor_copy(out=part[:], in_=ps4[:])
            nc.sync.dma_start(out=parts_d[:], in_=part[:])

    nc.compile()
    return nc


def _prepare(feats, labels, demog):
    """Host prep: balanced group binning, per-core shards, weights."""
    feats = np.ascontiguousarray(np.asarray(feats), dtype=np.float32)
    labels = np.asarray(labels).astype(np.int64)
    demog = np.asarray(demog).astype(np.int64)

    seg = (demog * 512 + labels).astype(np.int64)
    cnt = np.bincount(seg, minlength=G).astype(np.int64)

    # LPT: bins of exactly 128 groups, balanced by total row count
    import heapq
    order = np.argsort(-cnt, kind="stable")
    heap = [(0, b) for b in range(NB)]
    heapq.heapify(heap)
    bin_cnt = [0] * NB
    bin_groups = [[] for _ in range(NB)]
    bin_of_group = np.empty(G, np.int64)
    slot_of_group = np.empty(G, np.int64)
    for g in order:
        while True:
            tot, b = heapq.heappop(heap)
            if bin_cnt[b] < P:
                break
        bin_of_group[g] = b
        slot_of_group[g] = bin_cnt[b]
        bin_groups[b].append(g)
        bin_cnt[b] += 1
        if bin_cnt[b] < P:
            heapq.heappush(heap, (tot + int(cnt[g]), b))
    # swap refinement: move bin totals to the exact common mean when
    # possible (T_h drops by one tile when every bin hits the mean)
    tot = np.array([int(cnt[bin_of_group == b].sum()) for b in range(NB)],
                   np.int64)
    target = int(tot.sum()) // NB
    if tot.sum() == target * NB:
        for _ in range(400):
            if tot.max() == target and tot.min() == target:
                break
            A = int(np.argmax(tot))
            B = int(np.argmin(tot))
            want = (tot[A] - tot[B]) // 2
            gA = np.asarray(bin_groups[A])
            gB = np.asarray(bin_groups[B])
            diff = cnt[gA][:, None] - cnt[gB][None, :]
            i, k = np.unravel_index(np.argmin(np.abs(diff - want)), diff.shape)
            d = int(diff[i, k])
            if d <= 0:
                break
            g1, g2 = int(gA[i]), int(gB[k])
            bin_groups[A][i], bin_groups[B][k] = g2, g1
            bin_of_group[g1], bin_of_group[g2] = B, A
            s1, s2 = slot_of_group[g1], slot_of_group[g2]
            slot_of_group[g1], slot_of_group[g2] = s2, s1
            tot[A] -= d
            tot[B] += d
    T_h = max(1, -(-int(tot.max()) // P))

    pres = (cnt > 0)
    den = np.maximum(pres.reshape(ND, 512).sum(1), 1).astype(np.float64)
    W64 = np.where(pres, 1.0, 0.0) / (np.maximum(cnt, 1)
                                      * den[np.arange(G) // 512])
    invc64 = 1.0 / np.maximum(cnt, 1)

    f16 = feats.astype(np.float16)
    b_row = bin_of_group[seg]
    s_row = slot_of_group[seg]

    S_half = T_h * P
    in_maps = []
    for c in range(N_CORES):
        F = np.zeros((2 * S_half, D), np.float16)
        L = np.full((2 * S_half,), float(P), np.float32)
        aux = np.zeros((P, 10), np.float32)
        for k in (0, 1):
            b = 2 * c + k
            r = np.flatnonzero(b_row == b)
            off = k * S_half
            F[off:off + len(r)] = f16[r]
            L[off:off + len(r)] = s_row[r]
            gs = np.asarray(bin_groups[b], np.int64)
            aux[np.arange(len(gs)), 4 * k + gs // 512] = W64[gs]
            aux[np.arange(len(gs)), 8 + k] = invc64[gs]
        Ft = np.ascontiguousarray(
            F.reshape(2 * T_h, P, D).transpose(1, 0, 2)).reshape(P, -1)
        Lt = np.ascontiguousarray(L.reshape(2 * T_h, P).T)
        in_maps.append({"feats": Ft, "labs": Lt, "aux": aux})
    return T_h, in_maps


def _finish(res) -> np.float32:
    parts = np.zeros(ND, np.float64)
    for c in range(N_CORES):
        parts += np.asarray(res.results[c]["partials"],
                            np.float64).reshape(ND)
    mu = parts.mean()
    return np.float32(np.abs(parts - mu).mean())


def kernel(feats, labels, demog_labels, _results_out=None):
    T_h, in_maps = _prepare(feats, labels, demog_labels)
    key = (T_h, PAT)
    nc = _cache.get(key)
    if nc is None:
        nc = _cache.setdefault(key, _build(T_h))
    res = None
    last_exc = None
    for attempt in range(3):
        try:
            res = bass_utils.run_bass_kernel_spmd(
                nc, in_maps, core_ids=list(range(N_CORES)))
            break
        except Exception as e:  # transient axon worker hangups
            last_exc = e
            import time
            time.sleep(10)
    if res is None:
        raise last_exc
    if _results_out is not None:
        _results_out.append(res)
    return _finish(res)


# revision 15
# speedup vs baseline: 1.0509x; 1.0509x over previous
"""Trainium2 Bass kernel for nn_DebiasIntraDist (segment_reduce).

Full-input contract: kernel(**inputs) takes the complete (unsharded) inputs
and returns the full scalar loss.

Design (v3):
  * The 2048 (demog, label) groups are host-partitioned into 16 bins of
    exactly 128 groups each, balanced by row count (LPT), and each core
    processes two bins sequentially (2 "passes"). Each pass owns 128 groups,
    so the per-tile one-hot is a single [128, 128] stationary operand and
    every feature tile streams through the PE exactly once.
  * feats are quantized to fp16 on the host - rel error of the final loss
    vs the fp32 reference is ~1.2e-3 (measured on the fixed key-0 data),
    well inside the 2e-2 gate - which halves both DMA bytes and PE cost
    vs fp32, and (unlike bf16) makes a single fp16 column of the row
    sum-of-squares accurate enough, so no error-compensated hi/lo packs
    are needed anywhere.
  * Per-group count / presence / weights depend only on the labels, so they
    are computed on the host and folded into a per-slot weight matrix W:
        intra_d = sum_g W[g, d] * (sumsq_g - |sums_g|^2 / cnt_g)
    The device reduces its 256 groups to a [1, 4] per-demog partial with one
    tiny fp32 matmul per pass; the host sums the 8 cores' partials (32
    floats) and finishes the scalar loss. No collectives at all.
  * Row sum-of-squares work alternates between the Scalar engine
    (activation Square with accumulate) and the Vector+PE pair (x*x
    multiply, one-hot matmul into a second PSUM bank) with a compile-time
    pattern, so no single engine bottlenecks. GpSimd is kept out of the
    per-tile loop entirely (its tensor ops are ~1.7us each - software).
"""

import numpy as np

try:
    import concourse.bacc as bacc
except ImportError:  # fresh environment without PYTHONPATH set up
    import sys
    for p in ("/root/.axon_site/_ro/trn_rl_repo", "/opt/trn_rl_repo",
              "/root/.axon_site/_ro/pypackages"):
        if p not in sys.path:
            sys.path.append(p)
    import concourse.bacc as bacc
import concourse.mybir as mybir
import concourse.tile as tile
import concourse.bass_utils as bass_utils

N_CORES = 8
P = 128
D = 512          # feature dim
G = 2048         # total groups
NB = 16          # group bins (2 per core)
ND = 4           # demog values
CH = 8           # sample-tiles per steady-state feats DMA chunk

# per-(within-pass-tile) row-sumsq path pattern:
#   S = scalar activation square+accum -> fp16 sq column -> tiny PE matmul
#   P = vector x*x multiply -> PE one-hot matmul into a PSUM bank
PAT = "PSPSPSPSPSPSPSP"

_cache: dict[tuple, object] = {}


def _build(T_h: int, pat: str = PAT):
    """Compile the SPMD kernel: 2 passes of T_h tiles, 128 groups each."""
    T2 = 2 * T_h
    fp32 = mybir.dt.float32
    fp16 = mybir.dt.float16
    Alu = mybir.AluOpType
    Act = mybir.ActivationFunctionType

    paths = [pat[j % len(pat)] for j in range(T_h)]
    sq_tiles = [j for j in range(T_h) if paths[j] == "S"]   # feed ps_sm
    x2_tiles = [j for j in range(T_h) if paths[j] == "P"]   # feed ps_x2

    nc = bacc.Bacc("TRN2", target_bir_lowering=False, debug=False,
                   enable_asserts=True, num_devices=N_CORES)

    feats_d = nc.dram_tensor("feats", [P, T2 * D], fp16,
                             kind="ExternalInput").ap()
    labs_d = nc.dram_tensor("labs", [P, T2], fp32, kind="ExternalInput").ap()
    aux_d = nc.dram_tensor("aux", [P, 10], fp32, kind="ExternalInput").ap()
    parts_d = nc.dram_tensor("partials", [1, ND], fp32,
                             kind="ExternalOutput").ap()

    with tile.TileContext(nc) as tc:
        with (
            tc.tile_pool(name="const", bufs=1) as constp,
            tc.tile_pool(name="fx", bufs=4) as fxp,
            tc.tile_pool(name="oh", bufs=6) as ohp,
            tc.tile_pool(name="x2", bufs=6) as x2p,
            tc.tile_pool(name="scr", bufs=3) as scrp,
            tc.tile_pool(name="sq", bufs=6) as sqp,
            tc.tile_pool(name="post", bufs=1) as postp,
            tc.tile_pool(name="ps", bufs=1, space="PSUM") as psp,
        ):
            # constants / inputs (early, on queues that boot fastest)
            iota = constp.tile([P, P], fp16, tag="iota")
            nc.gpsimd.iota(iota[:], [[1, P]], channel_multiplier=0,
                           allow_small_or_imprecise_dtypes=True)
            labs = constp.tile([P, T2], fp32, tag="labs")
            auxs = constp.tile([P, 10], fp32, tag="auxs")

            # per-pass group accumulators (each owns a PSUM bank)
            ps_sums = [psp.tile([P, D], fp32, tag=f"sums{p}", name=f"sums{p}")
                       for p in range(2)]
            ps_x2 = [psp.tile([P, D], fp32, tag=f"x2{p}", name=f"x2{p}")
                     for p in range(2)] if x2_tiles else None
            ps_sm = [psp.tile([P, 1], fp32, tag=f"sm{p}", name=f"sm{p}")
                     for p in range(2)] if sq_tiles else None
            ps4 = psp.tile([1, ND], fp32, tag="p4", name="p4")

            def post_pass(p):
                """Reduce pass p's accumulators to a [1,4] demog partial."""
                ssq = postp.tile([P, 1], fp32, tag=f"ssq{p}")
                if sq_tiles:
                    nc.vector.tensor_copy(out=ssq[:], in_=ps_sm[p][:])
                    if x2_tiles:
                        ssq2 = postp.tile([P, 1], fp32, tag=f"ssq2{p}")
                        nc.vector.tensor_reduce(out=ssq2[:], in_=ps_x2[p][:],
                                                axis=mybir.AxisListType.X,
                                                op=Alu.add)
                        nc.vector.tensor_tensor(out=ssq[:], in0=ssq[:],
                                                in1=ssq2[:], op=Alu.add)
                else:
                    nc.vector.tensor_reduce(out=ssq[:], in_=ps_x2[p][:],
                                            axis=mybir.AxisListType.X,
                                            op=Alu.add)
                scr2 = scrp.tile([P, D], fp16, tag="scr")
                norm2 = postp.tile([P, 1], fp32, tag=f"n2{p}")
                nc.scalar.activation(scr2[:], ps_sums[p][:], Act.Square,
                                     accum_out=norm2[:])
                t1 = postp.tile([P, 1], fp32, tag=f"t1{p}")
                nc.vector.tensor_tensor(out=t1[:], in0=norm2[:],
                                        in1=auxs[:, 8 + p:9 + p], op=Alu.mult)
                val = postp.tile([P, 1], fp32, tag=f"val{p}")
                nc.vector.tensor_tensor(out=val[:], in0=ssq[:], in1=t1[:],
                                        op=Alu.subtract)
                nc.tensor.matmul(out=ps4[:], lhsT=val[:],
                                 rhs=auxs[:, 4 * p:4 * p + 4],
                                 start=(p == 0), stop=(p == 1))

            # chunked feats DMA schedule: ramped small first chunks so tile 0
            # is never queued behind megabytes of steady-state traffic; later
            # chunks are throttled by fx-pool buffer rotation (bufs=4)
            sched = []
            t = 0
            for want in (1, 1, 2, 3):
                if t < T2:
                    L = min(want, T2 - t)
                    sched.append((t, L))
                    t += L
            while t < T2:
                L = min(CH, T2 - t)
                sched.append((t, L))
                t += L

            ci = 0
            n_ramp = sum(1 for _, L in sched if L < CH)
            for t0, L in sched:
                fx = fxp.tile([P, CH * D], fp16, tag="fx")
                # chunk 0 alone on the scalar queue; the other ramp chunks
                # serialize their descriptor generation on the (otherwise
                # idle) gpsimd queue so tile 0's data never contends in the
                # DMA rings; steady chunks go to sync, which is naturally
                # throttled by fx-pool buffer rotation
                if ci == 0:
                    q = nc.scalar
                elif ci < n_ramp:
                    q = nc.gpsimd
                else:
                    q = nc.sync
                q.dma_start(out=fx[:, :L * D],
                            in_=feats_d[:, t0 * D:(t0 + L) * D])
                if ci == 0:
                    # labs right after tile 0 on the scalar queue (needed by
                    # the first is_equal)
                    nc.scalar.dma_start(out=labs[:], in_=labs_d[:])
                if ci == n_ramp - 1:
                    nc.gpsimd.dma_start(out=auxs[:], in_=aux_d[:])
                ci += 1
                for k in range(L):
                    ti = t0 + k
                    p, j = divmod(ti, T_h)
                    path = paths[j]
                    X = fx[:, k * D:(k + 1) * D]

                    if path == "P":
                        x2 = x2p.tile([P, D], fp16, tag="x2")
                        nc.vector.tensor_tensor(out=x2[:], in0=X, in1=X,
                                                op=Alu.mult)
                    oh = ohp.tile([P, P], fp16, tag="oh")
                    nc.vector.tensor_scalar(
                        out=oh[:], in0=iota[:], scalar1=labs[:, ti:ti + 1],
                        scalar2=None, op0=Alu.is_equal,
                    )
                    nc.tensor.matmul(out=ps_sums[p][:], lhsT=oh[:], rhs=X,
                                     start=(j == 0), stop=(j == T_h - 1))

                    if path == "P":
                        nc.tensor.matmul(out=ps_x2[p][:], lhsT=oh[:],
                                         rhs=x2[:],
                                         start=(j == x2_tiles[0]),
                                         stop=(j == x2_tiles[-1]))
                    else:
                        scr = scrp.tile([P, D], fp16, tag="scr")
                        sq = sqp.tile([P, 1], fp32, tag="sq")
                        nc.scalar.activation(scr[:], X, Act.Square,
                                             accum_out=sq[:])
                        sqh = sqp.tile([P, 1], fp16, tag="sqh")
                        nc.vector.tensor_copy(out=sqh[:], in_=sq[:])
                        nc.tensor.matmul(out=ps_sm[p][:], lhsT=oh[:],
                                         rhs=sqh[:],
                                         start=(j == sq_tiles[0]),
                                         stop=(j == sq_tiles[-1]))

                    if j == T_h - 1:
                        post_pass(p)

            part = postp.tile([1, ND], fp32, tag="part")
            nc.vector.tensor_copy(out=part[:], in_=ps4[:])
            nc.sync.dma_start(out=parts_d[:], in_=part[:])

    nc.compile()
    return nc


def _prepare(feats, labels, demog):
    """Host prep: balanced group binning, per-core shards, weights."""
    feats = np.ascontiguousarray(np.asarray(feats), dtype=np.float32)
    labels = np.asarray(labels).astype(np.int64)
    demog = np.asarray(demog).astype(np.int64)

    seg = (demog * 512 + labels).astype(np.int64)
    cnt = np.bincount(seg, minlength=G).astype(np.int64)

    # LPT: bins of exactly 128 groups, balanced by total row count
    import heapq
    order = np.argsort(-cnt, kind="stable")
    heap = [(0, b) for b in range(NB)]
    heapq.heapify(heap)
    bin_cnt = [0] * NB
    bin_groups = [[] for _ in range(NB)]
    bin_of_group = np.empty(G, np.int64)
    slot_of_group = np.empty(G, np.int64)
    for g in order:
        while True:
            tot, b = heapq.heappop(heap)
            if bin_cnt[b] < P:
                break
        bin_of_group[g] = b
        slot_of_group[g] = bin_cnt[b]
        bin_groups[b].append(g)
        bin_cnt[b] += 1
        if bin_cnt[b] < P:
            heapq.heappush(heap, (tot + int(cnt[g]), b))
    # swap refinement: move bin totals to the exact common mean when
    # possible (T_h drops by one tile when every bin hits the mean)
    tot = np.array([int(cnt[bin_of_group == b].sum()) for b in range(NB)],
                   np.int64)
    target = int(tot.sum()) // NB
    if tot.sum() == target * NB:
        for _ in range(400):
            if tot.max() == target and tot.min() == target:
                break
            A = int(np.argmax(tot))
            B = int(np.argmin(tot))
            want = (tot[A] - tot[B]) // 2
            gA = np.asarray(bin_groups[A])
            gB = np.asarray(bin_groups[B])
            diff = cnt[gA][:, None] - cnt[gB][None, :]
            i, k = np.unravel_index(np.argmin(np.abs(diff - want)), diff.shape)
            d = int(diff[i, k])
            if d <= 0:
                break
            g1, g2 = int(gA[i]), int(gB[k])
            bin_groups[A][i], bin_groups[B][k] = g2, g1
            bin_of_group[g1], bin_of_group[g2] = B, A
            s1, s2 = slot_of_group[g1], slot_of_group[g2]
            slot_of_group[g1], slot_of_group[g2] = s2, s1
            tot[A] -= d
            tot[B] += d
    T_h = max(1, -(-int(tot.max()) // P))

    pres = (cnt > 0)
    den = np.maximum(pres.reshape(ND, 512).sum(1), 1).astype(np.float64)
    W64 = np.where(pres, 1.0, 0.0) / (np.maximum(cnt, 1)
                                      * den[np.arange(G) // 512])
    invc64 = 1.0 / np.maximum(cnt, 1)

    f16 = feats.astype(np.float16)
    b_row = bin_of_group[seg]
    s_row = slot_of_group[seg]

    S_half = T_h * P
    in_maps = []
    for c in range(N_CORES):
        F = np.zeros((2 * S_half, D), np.float16)
        L = np.full((2 * S_half,), float(P), np.float32)
        aux = np.zeros((P, 10), np.float32)
        for k in (0, 1):
            b = 2 * c + k
            r = np.flatnonzero(b_row == b)
            off = k * S_half
            F[off:off + len(r)] = f16[r]
            L[off:off + len(r)] = s_row[r]
            gs = np.asarray(bin_groups[b], np.int64)
            aux[np.arange(len(gs)), 4 * k + gs // 512] = W64[gs]
            aux[np.arange(len(gs)), 8 + k] = invc64[gs]
        Ft = np.ascontiguousarray(
            F.reshape(2 * T_h, P, D).transpose(1, 0, 2)).reshape(P, -1)
        Lt = np.ascontiguousarray(L.reshape(2 * T_h, P).T)
        in_maps.append({"feats": Ft, "labs": Lt, "aux": aux})
    return T_h, in_maps


def _finish(res) -> np.float32:
    parts = np.zeros(ND, np.float64)
    for c in range(N_CORES):
        parts += np.asarray(res.results[c]["partials"],
                            np.float64).reshape(ND)
    mu = parts.mean()
    return np.float32(np.abs(parts - mu).mean())


def kernel(feats, labels, demog_labels, _results_out=None):
    T_h, in_maps = _prepare(feats, labels, demog_labels)
    key = (T_h, PAT)
    nc = _cache.get(key)
    if nc is None:
        nc = _cache.setdefault(key, _build(T_h))
    res = None
    last_exc = None
    for attempt in range(3):
        try:
            res = bass_utils.run_bass_kernel_spmd(
                nc, in_maps, core_ids=list(range(N_CORES)))
            break
        except Exception as e:  # transient axon worker hangups
            last_exc = e
            import time
            time.sleep(10)
    if res is None:
        raise last_exc
    if _results_out is not None:
        _results_out.append(res)
    return _finish(res)
